# revision 45
# baseline (speedup 1.0000x reference)
"""Multi-head attention (B=4, S=2048, D=768, 12 heads) on 8 TRN2 NeuronCores.

Sharding: data parallel over batch (4) x tensor parallel over heads (2 groups
of 6 heads) = 8 cores. Each core computes its (batch, head-group) slice:
  Q^T/K^T projections in [feat, seq] layout, V in [seq, feat] layout
  (augmented with a ones column per head so the P@V matmul also produces
  the softmax denominator), transposed scores S^T[k,q] per head pair with
  row-tiled K=64 matmuls into per-head PSUM tiles.

The softmax exp (B*H*S^2/8 = 25M elements/core) saturates the 1.2GHz
scalar engine (~164us alone), so it is split across TWO engines per step:
head0 on ACT, head1 on a custom single-pass DVE op
  exp(x*g) ~= (0.5*((x*g/8 + 1)^2 + 1))^8   (8 ALU stages, 1 elem/cy/lane)
split exactly at the head boundary so each PV matmul waits on exactly one
exp engine. Output leaves unnormalized (O rows + denominator row, bf16)
via 2x-rate DVE copies; the softmax divide happens on the host, removing
the reciprocal / broadcast-matmul / normalize-mul chain from block tails.
Projections drip into the PE stream by first-use deadline.
"""

import numpy as np
import ml_dtypes

B, S, D = 4, 2048, 768
NH, HD = 12, 64
HPC = 6                 # heads per core
FPC = HPC * HD          # 384 features per core
VW = HPC * (HD + 1)     # 390: V width with per-head ones column
N_CORES = 8
MT_CONST = 3            # head-pair tiles per core
BF16 = ml_dtypes.bfloat16

_PROGRAM = None

_EXP_OP = None


def _get_exp_op():
    """Custom DVE op: exp(x*g) ~= (0.5*((x*g/8 + 1)^2 + 1))^8, i.e.
    (1 + s + s^2/2)^8 with s = x*g/8 -- the quadratic Taylor of e^s plus
    three squarings. Exactly the 8 v3 ALU stages, one DVE pass/element.
    Log-domain error -(xg)^3/384, <2e-3 for this problem's |logits| < 2,
    and softmax renormalization cancels the common mode."""
    global _EXP_OP
    if _EXP_OP is not None:
        return _EXP_OP
    import concourse.dve_ops as dve_ops
    from concourse.dve_spec import Spec, Src0, C0, C1, C2, lower
    from concourse.dve_uop import DveOpSpec

    name = "EXP_POLY8_ANT"
    for op in dve_ops.OPS:
        if op.name == name:
            _EXP_OP = op
            return op

    u = Src0 * C0 + C1
    q = (u * u + C1) * C2
    q2 = q * q
    q4 = q2 * q2
    body = q4 * q4

    def ref(in0, in1, s0, s1, imm2):
        uu = in0.astype(np.float32) * np.float32(s0) + np.float32(s1)
        qq = (uu * uu + np.float32(s1)) * np.float32(imm2)
        return ((qq * qq) ** 2) ** 2

    spec = Spec(body=body, reference=ref)
    shas = {}
    for ver in ("v3", "v4"):
        try:
            uops = lower(spec, ver=ver)
        except Exception:
            continue
        shas[ver] = DveOpSpec(name=name, opcode=1, uops=uops,
                              rd1_en=False).sha(ver)
    row = max(dve_ops._SUB_OPCODE_FOR_NAME.values()) + 1
    assert row < 0x20, "no free DVE opcode row"
    op = dve_ops.DveOp(name, spec, subdim=False, uops_sha=shas)
    dve_ops.OPS.append(op)
    dve_ops._SUB_OPCODE_FOR_NAME[name] = row
    dve_ops.CUSTOM_DVE_SPECS[name] = spec
    _EXP_OP = op
    return op


def _emit_dve_exp(nc, out, in_, scale):
    """out = exp(in_ * scale), single DVE instruction."""
    op = _get_exp_op()
    with nc.allow_low_precision("poly exp in bf16 out"):
        return nc.vector._custom_dve(
            op, out=out, in0=in_, s0=scale / 8.0, s1=1.0, imm2=0.5)


def _build_program(repeats=1):
    import concourse.bass as bass  # noqa: F401
    import concourse.mybir as mybir
    from concourse import bacc
    from concourse.tile import TileContext
    from contextlib import ExitStack

    F = mybir.dt.float32
    BF = mybir.dt.bfloat16
    EXP = mybir.ActivationFunctionType.Exp

    nc = bacc.Bacc("TRN2", target_bir_lowering=False, debug=False, num_devices=N_CORES)

    xt = nc.dram_tensor("xt", [D, S], BF, kind="ExternalInput")
    wqt = nc.dram_tensor("wqt", [D, FPC], BF, kind="ExternalInput")
    wkt = nc.dram_tensor("wkt", [D, FPC], BF, kind="ExternalInput")
    wvt = nc.dram_tensor("wvt", [D, VW], BF, kind="ExternalInput")
    bqk = nc.dram_tensor("bqk", [FPC, 2], F, kind="ExternalInput")
    bv = nc.dram_tensor("bv", [1, VW], BF, kind="ExternalInput")
    # 6 blocks of [65, S]: 64 unnormalized O^T rows + the denominator row
    out = nc.dram_tensor("out", [MT_CONST * 2 * 65, S], BF, kind="ExternalOutput")

    KT = D // 128        # 6 contraction tiles for projections
    MT = FPC // 128      # 3 feature tiles (= head pairs)
    QC = S // 512        # 4 seq chunks of 512
    JT = S // 128        # 16 key tiles

    with TileContext(nc) as tc, ExitStack() as ctx:
        pools = {
            "const": ctx.enter_context(tc.tile_pool(name="const", bufs=1)),
            "qkv": ctx.enter_context(tc.tile_pool(name="qkv", bufs=1)),
            "osb": ctx.enter_context(tc.tile_pool(name="osb", bufs=1)),
            "pt": ctx.enter_context(tc.tile_pool(name="pt", bufs=6)),
            "small": ctx.enter_context(tc.tile_pool(name="small", bufs=4)),
            "pspr": ctx.enter_context(tc.tile_pool(name="pspr", bufs=2, space="PSUM")),
            "pss": ctx.enter_context(tc.tile_pool(name="pss", bufs=4, space="PSUM")),
            "pso": ctx.enter_context(tc.tile_pool(name="pso", bufs=1, space="PSUM")),
        }
        # ---- stage inputs in SBUF with one consolidated DMA per tensor
        # (many small dma_starts serialize ~0.65us each on the sequencer);
        # xt arrives per seq-chunk, interleaved with the projections that
        # consume each chunk
        const = pools["const"]
        small = pools["small"]
        xt_all = const.tile([128, KT * S], BF, tag="xta", name="xta")
        wq_all = const.tile([128, KT * FPC], BF, tag="wqa", name="wqa")
        wk_all = const.tile([128, KT * FPC], BF, tag="wka", name="wka")
        wv_all = const.tile([128, KT * VW], BF, tag="wva", name="wva")
        bqk_all = const.tile([128, MT * 2], F, tag="bqk", name="bqka")
        bv_s = const.tile([1, VW], BF, tag="bv")
        stage = {
            "xt_s": [xt_all[:, i * S:(i + 1) * S] for i in range(KT)],
            "wqt_s": [wq_all[:, i * FPC:(i + 1) * FPC] for i in range(KT)],
            "wkt_s": [wk_all[:, i * FPC:(i + 1) * FPC] for i in range(KT)],
            "wvt_s": [wv_all[:, i * VW:(i + 1) * VW] for i in range(KT)],
            "bq_s": [bqk_all[:, 2 * t_i:2 * t_i + 1] for t_i in range(MT)],
            "bk_s": [bqk_all[:, 2 * t_i + 1:2 * t_i + 2] for t_i in range(MT)],
            "bv_s": bv_s,
        }

        # DMA order follows first use: Q/K weights and xt chunk 0 gate the
        # first projections; V weights and biases are needed slightly later
        nc.sync.dma_start(
            wq_all[:].rearrange("p (b c) -> p b c", b=KT),
            wqt[:].rearrange("(b p) c -> p b c", p=128))
        nc.sync.dma_start(
            wk_all[:].rearrange("p (b c) -> p b c", b=KT),
            wkt[:].rearrange("(b p) c -> p b c", p=128))
        nc.sync.dma_start(
            bqk_all[:].rearrange("p (t c) -> p t c", t=MT),
            bqk[:].rearrange("(t p) c -> p t c", p=128))
        ones_s = const.tile([1, 128], BF, tag="ones")
        stage["ones_s"] = ones_s
        nc.vector.memset(ones_s[:], 1.0)

        # dummy exp so the ACT table set loads during the DMA prologue
        dummy = small.tile([1, 1], F, tag="dummy", name="dummy")
        nc.scalar.activation(dummy[:], stage["bq_s"][0][0:1, :], EXP)

        # xt chunk DMAs issued up front (DMA engines are otherwise idle);
        # chunk 0 lands first so the pair-0 chunk-0 projections can start
        # xt goes via SWDGE (gpsimd) so it runs in parallel with the weight
        # DMAs on the HWDGE ring
        for qc in range(QC):
            nc.gpsimd.dma_start(
                xt_all[:].rearrange("p (b c) -> p b c", b=KT)[
                    :, :, qc * 512:(qc + 1) * 512],
                xt[:].rearrange("(b p) c -> p b c", p=128)[
                    :, :, qc * 512:(qc + 1) * 512])
        nc.sync.dma_start(
            wv_all[:].rearrange("p (b c) -> p b c", b=KT),
            wvt[:].rearrange("(b p) c -> p b c", p=128))
        nc.sync.dma_start(bv_s[:], bv[:])

        for rep in range(repeats):
            _emit_body(nc, tc, pools, stage, mybir, F, BF, EXP, out,
                       KT, MT, QC, JT)

    nc.compile()
    return nc


def _emit_body(nc, tc, pools, stage, mybir, F, BF, EXP, out,
               KT, MT, QC, JT):
    if True:
        qkv = pools["qkv"]
        osb = pools["osb"]
        ppool = pools["pt"]
        small = pools["small"]
        ps_pr = pools["pspr"]
        ps_s = pools["pss"]
        ps_o = pools["pso"]
        xt_s = stage["xt_s"]
        wqt_s = stage["wqt_s"]
        wkt_s = stage["wkt_s"]
        wvt_s = stage["wvt_s"]
        bq_s = stage["bq_s"]
        bk_s = stage["bk_s"]
        bv_s = stage["bv_s"]
        ones_s = stage["ones_s"]

        v_s = [qkv.tile([128, VW], BF, tag=f"v{m}", name=f"v{m}") for m in range(JT)]
        qt_s = [qkv.tile([128, S], BF, tag=f"q{t_i}", name=f"qt{t_i}") for t_i in range(MT)]
        kt_s = [qkv.tile([128, S], BF, tag=f"k{t_i}", name=f"kt{t_i}") for t_i in range(MT)]
        o_s = [osb.tile([128, S], F, tag=f"o{t_i}", name=f"ot{t_i}") for t_i in range(MT)]

        # ---- projection work, broken into single-matmul thunks so the PE
        # stream can interleave them into the attention pipeline
        def v_group_thunks(m):
            # V projection (natural [seq, feat+ones] layout; the K=1 bias-row
            # matmul adds bv and the per-head ones column). use_act routes
            # the PSUM evacuation to the scalar engine (idle early on).
            cell = {}

            def mk(kk):
                def thunk():
                    if "ps" not in cell:
                        cell["ps"] = ps_pr.tile([128, VW], F, tag="pr",
                                                name=f"psv{m}")
                    if kk < KT:
                        nc.tensor.matmul(
                            cell["ps"][:],
                            lhsT=xt_s[kk][:, m * 128:(m + 1) * 128],
                            rhs=wvt_s[kk][:], start=(kk == 0), stop=False)
                    else:
                        nc.tensor.matmul(cell["ps"][:], lhsT=ones_s[:],
                                         rhs=bv_s[:], start=False, stop=True)
                        # ACT evacuation: the DVE is co-owner of the exp
                        # stream, keep its queue short
                        nc.scalar.copy(v_s[m][:], cell["ps"][:])
                return thunk
            return [mk(kk) for kk in range(KT + 1)]

        def qk_group_thunks(w_s, b_s, dst, p, qc):
            cell = {}

            def mk(kk):
                def thunk():
                    if "ps" not in cell:
                        cell["ps"] = ps_pr.tile([128, 512], F, tag="pr",
                                                name=f"psp{p}_{qc}")
                    nc.tensor.matmul(
                        cell["ps"][:],
                        lhsT=w_s[kk][:, p * 128:(p + 1) * 128],
                        rhs=xt_s[kk][:, qc * 512:(qc + 1) * 512],
                        start=(kk == 0), stop=(kk == KT - 1))
                    if kk == KT - 1:
                        nc.scalar.add(
                            dst[p][:, qc * 512:(qc + 1) * 512],
                            cell["ps"][:], b_s[p][:])
                return thunk
            return [mk(kk) for kk in range(KT)]

        # prologue compute: only what gates the very first attention step --
        # the pair-0 chunk-0 Q/K projections. Everything else drips into the
        # PE stream during the attention pipeline, ordered by when it is
        # first consumed (V tiles by k-step, K chunks early, Q chunks by
        # q-chunk, later pairs last).
        for w_s, b_s, dst in ((wqt_s, bq_s, qt_s), (wkt_s, bk_s, kt_s)):
            for th in qk_group_thunks(w_s, b_s, dst, 0, qc=0):
                th()

        # Tile tracks dependencies in EMISSION order, so every projection
        # thunk must be emitted strictly before its first consumer. Each
        # thunk gets a deadline (step index); the drip drains all due thunks
        # plus up to 3 more per step to smooth PE load.
        from collections import deque
        items = []   # (deadline, order, thunk)

        def add(deadline, thunks):
            for th in thunks:
                items.append((deadline, len(items), th))

        for m in range(JT):
            add(m, v_group_thunks(m))                       # PV(0,0,m) at step m
        for qc in range(1, QC):
            # kt chunk qc feeds scores(0,*,4qc..) first emitted at step 4qc-1
            add(max(0, 4 * qc - 2),
                qk_group_thunks(wkt_s, bk_s, kt_s, 0, qc))
            # qt chunk qc feeds scores(0,qc,0) first emitted at step 16qc-1
            add(max(0, 16 * qc - 2),
                qk_group_thunks(wqt_s, bq_s, qt_s, 0, qc))
        for p in range(1, MT):
            base = 64 * p
            for qc in range(QC):
                add(base + 4 * qc - 2,
                    qk_group_thunks(wkt_s, bk_s, kt_s, p, qc))
                add(base + 16 * qc - 2,
                    qk_group_thunks(wqt_s, bq_s, qt_s, p, qc))
        items.sort(key=lambda x: (x[0], x[1]))
        proj_q = deque(items)

        # ---- attention pipeline over flattened (pair, q-chunk, k-tile) steps
        steps = [(p, qc, j) for p in range(MT) for qc in range(QC)
                 for j in range(JT)]

        def emit_scores(p, qc, j):
            # per-head tiles: each exp releases its bank independently;
            # head1 first so the (slower) DVE exp gets its input earlier
            sps = [None, None]
            for h in (1, 0):
                sph = ps_s.tile([128, 512], F, tag="s", name=f"s{p}_{qc}_{j}_{h}")
                nc.tensor.matmul(
                    sph[:],
                    lhsT=kt_s[p][h * 64:(h + 1) * 64, j * 128:(j + 1) * 128],
                    rhs=qt_s[p][h * 64:(h + 1) * 64, qc * 512:(qc + 1) * 512],
                    start=True, stop=True, tile_position=(h * 64, 0))
                sps[h] = sph
            return sps

        # unnormalized O rows plus the ones-column denominator row go out
        # bf16 via a 2x-rate DVE copy and one DMA per (p, head); the final
        # softmax divide happens on the host (it needs a [q]-indexed
        # broadcast over partitions, which on-chip costs a PE broadcast
        # matmul + single-lane reciprocal per block)
        ost = [osb.tile([65, S], BF, tag=f"os{p}_{h}", name=f"os{p}_{h}")
               for p in range(MT) for h in range(2)]

        sp_next = emit_scores(*steps[0])
        Os = None
        for s, (p, qc, j) in enumerate(steps):
            sp = sp_next
            if s + 1 < len(steps):
                sp_next = emit_scores(*steps[s + 1])
            # due projection thunks first (correctness: V tiles feed this
            # step's PV, K/Q chunks feed the next scores emission)
            while proj_q and proj_q[0][0] <= s:
                proj_q.popleft()[2]()
            if j == 0:
                O0 = ps_o.tile([65, 512], F, tag="o0", name=f"o0_{p}_{qc}")
                O1 = ps_o.tile([65, 512], F, tag="o1", name=f"o1_{p}_{qc}")
                Os = (O0, O1)
            # exp ahead of the smoothing drip: the drip's DVE/ACT psum
            # evacuations have slack, the exp gates this step's PV
            pt = ppool.tile([128, 1024], BF, tag="p", name=f"pt{s}")
            _emit_dve_exp(nc, pt[:, 512:1024], sp[1][:], 0.125)
            nc.scalar.activation(pt[:, 0:512], sp[0][:], EXP, scale=0.125)
            for h in range(2):
                lh = 2 * p + h
                nc.tensor.matmul(
                    Os[h][:],
                    lhsT=v_s[j][:, lh * 65:(lh + 1) * 65],
                    rhs=pt[:, h * 512:(h + 1) * 512],
                    start=(j == 0), stop=(j == JT - 1))
            # one ahead-of-deadline thunk to smooth PE load
            extra = 1
            while proj_q and extra > 0:
                extra -= 1
                proj_q.popleft()[2]()
            if j == JT - 1:
                for h in range(2):
                    dst = ost[p * 2 + h]
                    with nc.allow_low_precision("unnormalized O in bf16"):
                        nc.vector.tensor_copy(
                            dst[:, qc * 512:(qc + 1) * 512], Os[h][:])
                    if qc == QC - 1:
                        nc.sync.dma_start(
                            out[(p * 2 + h) * 65:(p * 2 + h + 1) * 65, :],
                            dst[:])


def _get_program():
    global _PROGRAM
    if _PROGRAM is None:
        _PROGRAM = _build_program()
    return _PROGRAM


def _prep_core_inputs(inputs, Wq, bq, Wk, bk, Wv, bv, core):
    b, g = divmod(core, 2)
    hs = slice(g * FPC, (g + 1) * FPC)
    xt = np.ascontiguousarray(inputs[b].T).astype(BF16)
    wqt = np.ascontiguousarray(Wq[hs, :].T).astype(BF16)
    wkt = np.ascontiguousarray(Wk[hs, :].T).astype(BF16)
    wvt = np.zeros((D, VW), dtype=BF16)
    bv_aug = np.zeros((1, VW), dtype=BF16)
    for l in range(HPC):
        gh = g * HPC + l
        wvt[:, l * 65:l * 65 + 64] = Wv[gh * 64:(gh + 1) * 64, :].T.astype(BF16)
        bv_aug[0, l * 65:l * 65 + 64] = bv[gh * 64:(gh + 1) * 64].astype(BF16)
        bv_aug[0, l * 65 + 64] = 1.0
    bqk = np.stack([np.asarray(bq[hs], dtype=np.float32),
                    np.asarray(bk[hs], dtype=np.float32)], axis=1)
    return {
        "xt": xt,
        "wqt": wqt,
        "wkt": wkt,
        "wvt": wvt,
        "bqk": np.ascontiguousarray(bqk),
        "bv": bv_aug,
    }


def kernel(inputs, Wq, bq, Wk, bk, Wv, bv, _trace=False):
    from concourse.bass_utils import run_bass_kernel_spmd

    inputs = np.asarray(inputs, dtype=np.float32)
    Wq, Wk, Wv = (np.asarray(w, dtype=np.float32) for w in (Wq, Wk, Wv))
    bq, bk, bv = (np.asarray(b, dtype=np.float32) for b in (bq, bk, bv))
    in_maps = [
        _prep_core_inputs(inputs, Wq, bq, Wk, bk, Wv, bv, c) for c in range(N_CORES)
    ]
    nc = _get_program()
    res = run_bass_kernel_spmd(nc, in_maps, list(range(N_CORES)), trace=_trace)
    full = np.empty((B, S, D), dtype=np.float32)
    for c in range(N_CORES):
        b, g = divmod(c, 2)
        o = res.results[c]["out"].astype(np.float32)   # [390, S]
        for p in range(MT_CONST):
            for h in range(2):
                blk = o[(p * 2 + h) * 65:(p * 2 + h + 1) * 65]
                f0 = g * FPC + p * 128 + h * 64
                full[b, :, f0:f0 + 64] = (blk[0:64] / blk[64:65]).T
    if _trace:
        return full, res
    return full



# revision 46
# speedup vs baseline: 1.0730x; 1.0730x over previous
"""Multi-head attention (B=4, S=2048, D=768, 12 heads) on 8 TRN2 NeuronCores.

Sharding: data parallel over batch (4) x tensor parallel over heads (2 groups
of 6 heads) = 8 cores. Each core computes its (batch, head-group) slice:
  Q^T/K^T projections in [feat, seq] layout, V in [seq, feat] layout
  (augmented with a ones column per head so the P@V matmul also produces
  the softmax denominator), transposed scores S^T[k,q] per head pair with
  row-tiled K=64 matmuls into per-head PSUM tiles.

The softmax exp (B*H*S^2/8 = 25M elements/core) saturates the 1.2GHz
scalar engine (~164us alone), so it is split across TWO engines per step:
head0 on ACT, head1 on a custom single-pass DVE op
  exp(x*g) ~= (0.5*((x*g/8 + 1)^2 + 1))^8   (8 ALU stages, 1 elem/cy/lane)
split exactly at the head boundary so each PV matmul waits on exactly one
exp engine. Output leaves unnormalized (O rows + denominator row, bf16)
via 2x-rate DVE copies; the softmax divide happens on the host, removing
the reciprocal / broadcast-matmul / normalize-mul chain from block tails.
Projections drip into the PE stream by first-use deadline.
"""

import numpy as np
import ml_dtypes

B, S, D = 4, 2048, 768
NH, HD = 12, 64
HPC = 6                 # heads per core
FPC = HPC * HD          # 384 features per core
VW = HPC * (HD + 1)     # 390: V width with per-head ones column
N_CORES = 8
MT_CONST = 3            # head-pair tiles per core
BF16 = ml_dtypes.bfloat16

_PROGRAM = None

_EXP_OP = None


def _get_exp_op():
    """Custom DVE op: exp(x*g) ~= (0.5*((x*g/8 + 1)^2 + 1))^8, i.e.
    (1 + s + s^2/2)^8 with s = x*g/8 -- the quadratic Taylor of e^s plus
    three squarings. Exactly the 8 v3 ALU stages, one DVE pass/element.
    Log-domain error -(xg)^3/384, <2e-3 for this problem's |logits| < 2,
    and softmax renormalization cancels the common mode."""
    global _EXP_OP
    if _EXP_OP is not None:
        return _EXP_OP
    import concourse.dve_ops as dve_ops
    from concourse.dve_spec import Spec, Src0, C0, C1, C2, lower
    from concourse.dve_uop import DveOpSpec

    name = "EXP_POLY8_ANT"
    for op in dve_ops.OPS:
        if op.name == name:
            _EXP_OP = op
            return op

    u = Src0 * C0 + C1
    q = (u * u + C1) * C2
    q2 = q * q
    q4 = q2 * q2
    body = q4 * q4

    def ref(in0, in1, s0, s1, imm2):
        uu = in0.astype(np.float32) * np.float32(s0) + np.float32(s1)
        qq = (uu * uu + np.float32(s1)) * np.float32(imm2)
        return ((qq * qq) ** 2) ** 2

    spec = Spec(body=body, reference=ref)
    shas = {}
    for ver in ("v3", "v4"):
        try:
            uops = lower(spec, ver=ver)
        except Exception:
            continue
        shas[ver] = DveOpSpec(name=name, opcode=1, uops=uops,
                              rd1_en=False).sha(ver)
    row = max(dve_ops._SUB_OPCODE_FOR_NAME.values()) + 1
    assert row < 0x20, "no free DVE opcode row"
    op = dve_ops.DveOp(name, spec, subdim=False, uops_sha=shas)
    dve_ops.OPS.append(op)
    dve_ops._SUB_OPCODE_FOR_NAME[name] = row
    dve_ops.CUSTOM_DVE_SPECS[name] = spec
    _EXP_OP = op
    return op


def _emit_dve_exp(nc, out, in_, scale):
    """out = exp(in_ * scale), single DVE instruction."""
    op = _get_exp_op()
    with nc.allow_low_precision("poly exp in bf16 out"):
        return nc.vector._custom_dve(
            op, out=out, in0=in_, s0=scale / 8.0, s1=1.0, imm2=0.5)


def _build_program(repeats=1):
    import concourse.bass as bass  # noqa: F401
    import concourse.mybir as mybir
    from concourse import bacc
    from concourse.tile import TileContext
    from contextlib import ExitStack

    F = mybir.dt.float32
    BF = mybir.dt.bfloat16
    EXP = mybir.ActivationFunctionType.Exp

    nc = bacc.Bacc("TRN2", target_bir_lowering=False, debug=False, num_devices=N_CORES)

    xt = nc.dram_tensor("xt", [D, S], BF, kind="ExternalInput")
    wqt = nc.dram_tensor("wqt", [D, FPC], BF, kind="ExternalInput")
    wkt = nc.dram_tensor("wkt", [D, FPC], BF, kind="ExternalInput")
    wvt = nc.dram_tensor("wvt", [D, VW], BF, kind="ExternalInput")
    bqk = nc.dram_tensor("bqk", [FPC, 2], F, kind="ExternalInput")
    bv = nc.dram_tensor("bv", [1, VW], BF, kind="ExternalInput")
    # 6 blocks of [65, S]: 64 unnormalized O^T rows + the denominator row
    out = nc.dram_tensor("out", [MT_CONST * 2 * 65, S], BF, kind="ExternalOutput")

    KT = D // 128        # 6 contraction tiles for projections
    MT = FPC // 128      # 3 feature tiles (= head pairs)
    QC = S // 512        # 4 seq chunks of 512
    JT = S // 128        # 16 key tiles

    with TileContext(nc) as tc, ExitStack() as ctx:
        pools = {
            "const": ctx.enter_context(tc.tile_pool(name="const", bufs=1)),
            "qkv": ctx.enter_context(tc.tile_pool(name="qkv", bufs=1)),
            "osb": ctx.enter_context(tc.tile_pool(name="osb", bufs=1)),
            "pt": ctx.enter_context(tc.tile_pool(name="pt", bufs=6)),
            "small": ctx.enter_context(tc.tile_pool(name="small", bufs=4)),
            "pspr": ctx.enter_context(tc.tile_pool(name="pspr", bufs=2, space="PSUM")),
            "pss": ctx.enter_context(tc.tile_pool(name="pss", bufs=4, space="PSUM")),
            "pso": ctx.enter_context(tc.tile_pool(name="pso", bufs=1, space="PSUM")),
        }
        # ---- stage inputs in SBUF with one consolidated DMA per tensor
        # (many small dma_starts serialize ~0.65us each on the sequencer);
        # xt arrives per seq-chunk, interleaved with the projections that
        # consume each chunk
        const = pools["const"]
        small = pools["small"]
        xt_all = const.tile([128, KT * S], BF, tag="xta", name="xta")
        wq_all = const.tile([128, KT * FPC], BF, tag="wqa", name="wqa")
        wk_all = const.tile([128, KT * FPC], BF, tag="wka", name="wka")
        wv_all = const.tile([128, KT * VW], BF, tag="wva", name="wva")
        bqk_all = const.tile([128, MT * 2], F, tag="bqk", name="bqka")
        bv_s = const.tile([1, VW], BF, tag="bv")
        stage = {
            "xt_s": [xt_all[:, i * S:(i + 1) * S] for i in range(KT)],
            "wqt_s": [wq_all[:, i * FPC:(i + 1) * FPC] for i in range(KT)],
            "wkt_s": [wk_all[:, i * FPC:(i + 1) * FPC] for i in range(KT)],
            "wvt_s": [wv_all[:, i * VW:(i + 1) * VW] for i in range(KT)],
            "bq_s": [bqk_all[:, 2 * t_i:2 * t_i + 1] for t_i in range(MT)],
            "bk_s": [bqk_all[:, 2 * t_i + 1:2 * t_i + 2] for t_i in range(MT)],
            "bv_s": bv_s,
        }

        # DMA order follows first use: Q/K weights and xt chunk 0 gate the
        # first projections; V weights and biases are needed slightly later
        nc.sync.dma_start(
            wq_all[:].rearrange("p (b c) -> p b c", b=KT),
            wqt[:].rearrange("(b p) c -> p b c", p=128))
        nc.sync.dma_start(
            wk_all[:].rearrange("p (b c) -> p b c", b=KT),
            wkt[:].rearrange("(b p) c -> p b c", p=128))
        nc.sync.dma_start(
            bqk_all[:].rearrange("p (t c) -> p t c", t=MT),
            bqk[:].rearrange("(t p) c -> p t c", p=128))
        ones_s = const.tile([1, 128], BF, tag="ones")
        stage["ones_s"] = ones_s
        nc.vector.memset(ones_s[:], 1.0)

        # dummy exp so the ACT table set loads during the DMA prologue
        dummy = small.tile([1, 1], F, tag="dummy", name="dummy")
        nc.scalar.activation(dummy[:], stage["bq_s"][0][0:1, :], EXP)

        # warm-up matmuls: the PE is data-starved during the input DMAs
        # and the HAM clock-gate needs ~3.4us of activity to reach full
        # rate; burn the wait on junk MMs (into one scores-pool rotation
        # slot, overwritten later) so the real prologue projections run
        # at full clock
        warm = pools["pss"].tile([128, 512], F, tag="s", name="warm")
        for _ in range(45):
            nc.tensor.matmul(warm[:, 0:128], lhsT=ones_s[:],
                             rhs=ones_s[:], start=True, stop=True)

        # xt chunk DMAs issued up front (DMA engines are otherwise idle);
        # chunk 0 lands first so the pair-0 chunk-0 projections can start
        # xt goes via SWDGE (gpsimd) so it runs in parallel with the weight
        # DMAs on the HWDGE ring
        for qc in range(QC):
            nc.gpsimd.dma_start(
                xt_all[:].rearrange("p (b c) -> p b c", b=KT)[
                    :, :, qc * 512:(qc + 1) * 512],
                xt[:].rearrange("(b p) c -> p b c", p=128)[
                    :, :, qc * 512:(qc + 1) * 512])
        nc.sync.dma_start(
            wv_all[:].rearrange("p (b c) -> p b c", b=KT),
            wvt[:].rearrange("(b p) c -> p b c", p=128))
        nc.sync.dma_start(bv_s[:], bv[:])

        for rep in range(repeats):
            _emit_body(nc, tc, pools, stage, mybir, F, BF, EXP, out,
                       KT, MT, QC, JT)

    nc.compile()
    return nc


def _emit_body(nc, tc, pools, stage, mybir, F, BF, EXP, out,
               KT, MT, QC, JT):
    if True:
        qkv = pools["qkv"]
        osb = pools["osb"]
        ppool = pools["pt"]
        small = pools["small"]
        ps_pr = pools["pspr"]
        ps_s = pools["pss"]
        ps_o = pools["pso"]
        xt_s = stage["xt_s"]
        wqt_s = stage["wqt_s"]
        wkt_s = stage["wkt_s"]
        wvt_s = stage["wvt_s"]
        bq_s = stage["bq_s"]
        bk_s = stage["bk_s"]
        bv_s = stage["bv_s"]
        ones_s = stage["ones_s"]

        v_s = [qkv.tile([128, VW], BF, tag=f"v{m}", name=f"v{m}") for m in range(JT)]
        qt_s = [qkv.tile([128, S], BF, tag=f"q{t_i}", name=f"qt{t_i}") for t_i in range(MT)]
        kt_s = [qkv.tile([128, S], BF, tag=f"k{t_i}", name=f"kt{t_i}") for t_i in range(MT)]
        o_s = [osb.tile([128, S], F, tag=f"o{t_i}", name=f"ot{t_i}") for t_i in range(MT)]

        # ---- projection work, broken into single-matmul thunks so the PE
        # stream can interleave them into the attention pipeline
        def v_group_thunks(m):
            # V projection (natural [seq, feat+ones] layout; the K=1 bias-row
            # matmul adds bv and the per-head ones column). use_act routes
            # the PSUM evacuation to the scalar engine (idle early on).
            cell = {}

            def mk(kk):
                def thunk():
                    if "ps" not in cell:
                        cell["ps"] = ps_pr.tile([128, VW], F, tag="pr",
                                                name=f"psv{m}")
                    if kk < KT:
                        nc.tensor.matmul(
                            cell["ps"][:],
                            lhsT=xt_s[kk][:, m * 128:(m + 1) * 128],
                            rhs=wvt_s[kk][:], start=(kk == 0), stop=False)
                    else:
                        nc.tensor.matmul(cell["ps"][:], lhsT=ones_s[:],
                                         rhs=bv_s[:], start=False, stop=True)
                        # ACT evacuation: the DVE is co-owner of the exp
                        # stream, keep its queue short
                        nc.scalar.copy(v_s[m][:], cell["ps"][:])
                return thunk
            return [mk(kk) for kk in range(KT + 1)]

        def qk_group_thunks(w_s, b_s, dst, p, qc):
            cell = {}

            def mk(kk):
                def thunk():
                    if "ps" not in cell:
                        cell["ps"] = ps_pr.tile([128, 512], F, tag="pr",
                                                name=f"psp{p}_{qc}")
                    nc.tensor.matmul(
                        cell["ps"][:],
                        lhsT=w_s[kk][:, p * 128:(p + 1) * 128],
                        rhs=xt_s[kk][:, qc * 512:(qc + 1) * 512],
                        start=(kk == 0), stop=(kk == KT - 1))
                    if kk == KT - 1:
                        nc.scalar.add(
                            dst[p][:, qc * 512:(qc + 1) * 512],
                            cell["ps"][:], b_s[p][:])
                return thunk
            return [mk(kk) for kk in range(KT)]

        # prologue compute: only what gates the very first attention step --
        # the pair-0 chunk-0 Q/K projections. Everything else drips into the
        # PE stream during the attention pipeline, ordered by when it is
        # first consumed (V tiles by k-step, K chunks early, Q chunks by
        # q-chunk, later pairs last).
        for w_s, b_s, dst in ((wqt_s, bq_s, qt_s), (wkt_s, bk_s, kt_s)):
            for th in qk_group_thunks(w_s, b_s, dst, 0, qc=0):
                th()

        # Tile tracks dependencies in EMISSION order, so every projection
        # thunk must be emitted strictly before its first consumer. Each
        # thunk gets a deadline (step index); the drip drains all due thunks
        # plus up to 3 more per step to smooth PE load.
        from collections import deque
        items = []   # (deadline, order, thunk)

        def add(deadline, thunks):
            for th in thunks:
                items.append((deadline, len(items), th))

        for m in range(JT):
            add(m, v_group_thunks(m))                       # PV(0,0,m) at step m
        for qc in range(1, QC):
            # kt chunk qc feeds scores(0,*,4qc..) first emitted at step 4qc-1
            add(max(0, 4 * qc - 2),
                qk_group_thunks(wkt_s, bk_s, kt_s, 0, qc))
            # qt chunk qc feeds scores(0,qc,0) first emitted at step 16qc-1
            add(max(0, 16 * qc - 2),
                qk_group_thunks(wqt_s, bq_s, qt_s, 0, qc))
        for p in range(1, MT):
            base = 64 * p
            for qc in range(QC):
                add(base + 4 * qc - 2,
                    qk_group_thunks(wkt_s, bk_s, kt_s, p, qc))
                add(base + 16 * qc - 2,
                    qk_group_thunks(wqt_s, bq_s, qt_s, p, qc))
        items.sort(key=lambda x: (x[0], x[1]))
        proj_q = deque(items)

        # ---- attention pipeline over flattened (pair, q-chunk, k-tile) steps
        steps = [(p, qc, j) for p in range(MT) for qc in range(QC)
                 for j in range(JT)]

        def emit_scores(p, qc, j):
            # per-head tiles: each exp releases its bank independently;
            # head1 first so the (slower) DVE exp gets its input earlier
            sps = [None, None]
            for h in (1, 0):
                sph = ps_s.tile([128, 512], F, tag="s", name=f"s{p}_{qc}_{j}_{h}")
                nc.tensor.matmul(
                    sph[:],
                    lhsT=kt_s[p][h * 64:(h + 1) * 64, j * 128:(j + 1) * 128],
                    rhs=qt_s[p][h * 64:(h + 1) * 64, qc * 512:(qc + 1) * 512],
                    start=True, stop=True, tile_position=(h * 64, 0))
                sps[h] = sph
            return sps

        # unnormalized O rows plus the ones-column denominator row go out
        # bf16 via a 2x-rate DVE copy and one DMA per (p, head); the final
        # softmax divide happens on the host (it needs a [q]-indexed
        # broadcast over partitions, which on-chip costs a PE broadcast
        # matmul + single-lane reciprocal per block)
        ost = [osb.tile([65, S], BF, tag=f"os{p}_{h}", name=f"os{p}_{h}")
               for p in range(MT) for h in range(2)]

        sp_next = emit_scores(*steps[0])
        Os = None
        for s, (p, qc, j) in enumerate(steps):
            sp = sp_next
            if s + 1 < len(steps):
                sp_next = emit_scores(*steps[s + 1])
            # due projection thunks first (correctness: V tiles feed this
            # step's PV, K/Q chunks feed the next scores emission)
            while proj_q and proj_q[0][0] <= s:
                proj_q.popleft()[2]()
            if j == 0:
                O0 = ps_o.tile([65, 512], F, tag="o0", name=f"o0_{p}_{qc}")
                O1 = ps_o.tile([65, 512], F, tag="o1", name=f"o1_{p}_{qc}")
                Os = (O0, O1)
            # exp ahead of the smoothing drip: the drip's DVE/ACT psum
            # evacuations have slack, the exp gates this step's PV
            pt = ppool.tile([128, 1024], BF, tag="p", name=f"pt{s}")
            _emit_dve_exp(nc, pt[:, 512:1024], sp[1][:], 0.125)
            nc.scalar.activation(pt[:, 0:512], sp[0][:], EXP, scale=0.125)
            for h in range(2):
                lh = 2 * p + h
                nc.tensor.matmul(
                    Os[h][:],
                    lhsT=v_s[j][:, lh * 65:(lh + 1) * 65],
                    rhs=pt[:, h * 512:(h + 1) * 512],
                    start=(j == 0), stop=(j == JT - 1))
            # one ahead-of-deadline thunk to smooth PE load
            extra = 1
            while proj_q and extra > 0:
                extra -= 1
                proj_q.popleft()[2]()
            if j == JT - 1:
                for h in range(2):
                    dst = ost[p * 2 + h]
                    with nc.allow_low_precision("unnormalized O in bf16"):
                        nc.vector.tensor_copy(
                            dst[:, qc * 512:(qc + 1) * 512], Os[h][:])
                    if qc == QC - 1:
                        nc.sync.dma_start(
                            out[(p * 2 + h) * 65:(p * 2 + h + 1) * 65, :],
                            dst[:])


def _get_program():
    global _PROGRAM
    if _PROGRAM is None:
        _PROGRAM = _build_program()
    return _PROGRAM


def _prep_core_inputs(inputs, Wq, bq, Wk, bk, Wv, bv, core):
    b, g = divmod(core, 2)
    hs = slice(g * FPC, (g + 1) * FPC)
    xt = np.ascontiguousarray(inputs[b].T).astype(BF16)
    wqt = np.ascontiguousarray(Wq[hs, :].T).astype(BF16)
    wkt = np.ascontiguousarray(Wk[hs, :].T).astype(BF16)
    wvt = np.zeros((D, VW), dtype=BF16)
    bv_aug = np.zeros((1, VW), dtype=BF16)
    for l in range(HPC):
        gh = g * HPC + l
        wvt[:, l * 65:l * 65 + 64] = Wv[gh * 64:(gh + 1) * 64, :].T.astype(BF16)
        bv_aug[0, l * 65:l * 65 + 64] = bv[gh * 64:(gh + 1) * 64].astype(BF16)
        bv_aug[0, l * 65 + 64] = 1.0
    bqk = np.stack([np.asarray(bq[hs], dtype=np.float32),
                    np.asarray(bk[hs], dtype=np.float32)], axis=1)
    return {
        "xt": xt,
        "wqt": wqt,
        "wkt": wkt,
        "wvt": wvt,
        "bqk": np.ascontiguousarray(bqk),
        "bv": bv_aug,
    }


def kernel(inputs, Wq, bq, Wk, bk, Wv, bv, _trace=False):
    from concourse.bass_utils import run_bass_kernel_spmd

    inputs = np.asarray(inputs, dtype=np.float32)
    Wq, Wk, Wv = (np.asarray(w, dtype=np.float32) for w in (Wq, Wk, Wv))
    bq, bk, bv = (np.asarray(b, dtype=np.float32) for b in (bq, bk, bv))
    in_maps = [
        _prep_core_inputs(inputs, Wq, bq, Wk, bk, Wv, bv, c) for c in range(N_CORES)
    ]
    nc = _get_program()
    res = run_bass_kernel_spmd(nc, in_maps, list(range(N_CORES)), trace=_trace)
    full = np.empty((B, S, D), dtype=np.float32)
    for c in range(N_CORES):
        b, g = divmod(c, 2)
        o = res.results[c]["out"].astype(np.float32)   # [390, S]
        for p in range(MT_CONST):
            for h in range(2):
                blk = o[(p * 2 + h) * 65:(p * 2 + h + 1) * 65]
                f0 = g * FPC + p * 128 + h * 64
                full[b, :, f0:f0 + 64] = (blk[0:64] / blk[64:65]).T
    if _trace:
        return full, res
    return full

